# revision 23
# baseline (speedup 1.0000x reference)
"""BBox spatial attention kernel for Trainium2 (8 NeuronCores, data-parallel over B).

Reference math per batch b, box n:
    gauss[n, y, x] = exp(-(dy2[n, y] + dx2[n, x]))
    att[y, x]      = max_n gauss      (all-zero boxes masked out)

Because each gauss plane is rank-1 separable, the p-norm power trick turns the
max into ONE K=32 matmul per batch per power level:
    S_p[y, x] = sum_n (C g^p)[n, y] * (C g^p)[n, x],  (sum g^p)^(1/p) -> max g
(product scale 2^58 keeps S <= 2^63: measured on HW, the Scalar-Engine Ln is
exact on [2^-64, 2^64], saturates to -45.8614 below 2^-64, and returns
garbage above 2^64.)
Levels p = 28, 56 are combined with a 2-term Richardson correction
(exact for two-way ties):  m^112 = S56 * (1 + sqrt(1 - (S28^2/S56 - 1)^2)) / 2,
evaluated in log space with a quadratic fit of
g(D) = ln((1+sqrt(1-(e^D-1)^2))/2) on D in [0, ln2].  A min-cascade over the
level estimates (p-norm monotonicity: each level over-estimates, deeper
levels are tighter but underflow earlier) picks the deepest alive level per
pixel with a plain elementwise min - no selects:
    out = exp(min(2*L28, L56 + g(D), 4*L14 - 3*40.20) / 56 - 40.20/56)
The shallow p = 14 level covers the far field.  The HW Ln saturation value
-45.8614 acts as a free dead-level floor: a level whose S drops below 2^-64
reports its window-cliff bound (>= truth there), so the min discards dead
levels; the p=14 floor exp(-6.15) = 0.0021 bounds the far-field error.
e14 factors carry 2^29 (so S14 reaches deep); e28 carries 2^14.5 so that
e56 = e28^2 (one DVE op, no ACT exp) lands at the same 2^29 factor scale.
Validated vs the fp64 reference on the seed-0 inputs: rel err 1.11e-2 (tol 2e-2).

All activation funcs used (Exp, Ln) live in the natural_log_exp_and_others
table; _collapse_act_table_loads repoints the greedily-inserted per-rep
table reloads (2 x 1.28us) at that single set.

The body is emitted as a 3-stage software pipeline D(k+2) A(k+1) B(k)
(D: input DMA, A: params/factors/matmuls, B: ln/cascade/exp/output) so each
in-order engine stream only meets ready operands: the input-DMA issue+sem
latency (~2.3us) and the PE->Ln dependency are hidden by neighboring reps.

Layout: partitions = (b n) [64 rows], free = [y-block 0:128 | x-block 128:256].
All-zero boxes get a big penalty folded into the Exp bias vector -> factors 0.

Sharding: B=16 -> 2 batches per core, 8 cores, no cross-core comms.
feature_map only provides H/W and is never touched.
"""

import math

import numpy as np

import concourse.bacc as bacc
import concourse.bass as bass
import concourse.mybir as mybir
import concourse.tile as tile
from concourse.bass_utils import run_bass_kernel_spmd

B, N, H, W = 16, 32, 128, 128
N_CORES = 8
B_LOC = B // N_CORES  # 2 batches per core
EPS = 1e-6
F32 = mybir.dt.float32
F16 = mybir.dt.float16
BF16 = mybir.dt.bfloat16
ALU = mybir.AluOpType
ACT = mybir.ActivationFunctionType

MAGIC = 8388608.0  # 2^23
LN2 = math.log(2.0)
CL = 29 * LN2  # ln 2^29, the e14 factor scale
K58 = 58 * LN2  # ln 2^58, the product scale
LNB = 1e-30  # uniform Ln bias: avoids ln(0) -> -inf; HW floors it to -45.8614
PC1 = 0.967875  # g(D) ~ PC2*D^2 + PC1*D, pinned at g(ln2) = -ln2
PC2 = -2.839043
BK = 512  # PSUM bank, in f32 columns

_CACHE: dict = {}


def build_nc(reps: int = 1):
    nc = bacc.Bacc(
        "TRN2",
        target_bir_lowering=False,
        debug=False,
        enable_asserts=False,
    )
    bb = nc.dram_tensor("bb", [B_LOC, N, 4], F32, kind="ExternalInput")
    att = nc.dram_tensor("att", [B_LOC, H, W], F32, kind="ExternalOutput")
    iota2_dram = nc.inline_tensor(
        np.tile(2.0 * np.arange(W, dtype=np.float32), (2 * N, 1)), name="iota2_const"
    )

    with tile.TileContext(nc) as tc:
        with (
            tc.tile_pool(name="sb", bufs=2) as sb,
            tc.tile_pool(name="cst", bufs=1) as cst,
            tc.tile_pool(name="psum", bufs=1, space="PSUM") as pp,
        ):
            # one-time: ACT table warm at t=0, const loads
            warm = cst.tile([128, 1], F32, tag="warm")
            nc.vector.memset(warm[:], 1.0)
            nc.scalar.activation(warm[:], warm[:], ACT.Exp)
            nc.scalar.activation(warm[:], warm[:], ACT.Ln)
            iota2 = cst.tile([2 * N, W], F32, tag="iota2")
            nc.sync.dma_start(iota2[:], iota2_dram.ap())
            b0c = cst.tile([128, 1], F32, tag="b0c")
            nc.vector.memset(b0c[:], LNB)
            ebc = cst.tile([128, 1], F32, tag="ebc")
            nc.vector.memset(ebc[:], -K58 / 56.0)

            # 3-stage software pipeline: D(k+2) A(k+1) B(k)
            bbts = [_phase_d(nc, sb, bb)]
            sts = []
            if reps > 1:
                bbts.append(_phase_d(nc, sb, bb))
            sts.append(_phase_a(nc, sb, pp, bbts[0], iota2))
            for k in range(1, reps):
                if k + 1 < reps:
                    bbts.append(_phase_d(nc, sb, bb))
                sts.append(_phase_a(nc, sb, pp, bbts[k], iota2))
                _phase_b(nc, sb, att, sts[k - 1], b0c, ebc)
            _phase_b(nc, sb, att, sts[reps - 1], b0c, ebc)

    nc.compile()
    _collapse_act_table_loads(nc)
    return nc


def _collapse_act_table_loads(nc):
    """All activation funcs used here (Exp, Ln) live together in the
    natural_log_exp_and_others table, but the greedy insertion pass picks
    exp_and_others/natural_log alternately, reloading the table RAM twice
    per rep (~2.6 us).  Keep one load, pointed at the combined set."""
    from concourse.hw_specs import get_activation_tables

    names = list(get_activation_tables(nc.m.arch))
    combined = names.index("natural_log_exp_and_others")
    first = True
    for blk in nc.m.functions[0].blocks:
        keep = []
        for inst in blk.instructions:
            if isinstance(inst, mybir.InstLoadActFuncSet):
                if not first:
                    continue
                inst.act_func_set_id = combined
                first = False
            keep.append(inst)
        if len(keep) != len(blk.instructions):
            blk.instructions[:] = keep


def _phase_d(nc, sb, bb):
    # bbt[(b n), c]: c = (x1, y1, x2, y2)
    bbt = sb.tile([2 * N, 4], F32, tag="bbt", bufs=3)
    nc.sync.dma_start(bbt[:], bb.ap().rearrange("b n c -> (b n) c"))
    return bbt


def _phase_a(nc, sb, pp, bbt, iota2):
    # --- per-box params on Pool; columns (x, y) ---
    # pixel coords clip(floor(v*128), 0, 127) via round-half magic:
    # a = v*128 + (2^23 - 0.5) rounds RNE to 2^23 + floor (v in [0,1)).
    a = sb.tile([2 * N, 4], F32, tag="a")
    nc.gpsimd.tensor_scalar(a[:], bbt[:], float(W), MAGIC - 0.5, ALU.mult, ALU.add)
    bm = sb.tile([2 * N, 4], F32, tag="bm")
    nc.gpsimd.tensor_scalar(bm[:], a[:], MAGIC, -1.0, ALU.max, ALU.mult)
    # fn = -clip(floor), small magnitude (the 2^23 offsets cancel EXACTLY;
    # summing bm directly would round at the 2^24 boundary)
    fn = sb.tile([2 * N, 4], F32, tag="fn")
    nc.gpsimd.tensor_scalar(fn[:], bm[:], MAGIC, None, ALU.add)
    s = sb.tile([2 * N, 2], F32, tag="s")
    nc.gpsimd.tensor_tensor(s[:], bm[:, 0:2], bm[:, 2:4], ALU.subtract)  # hi-lo
    d = sb.tile([2 * N, 2], F32, tag="d")
    nc.gpsimd.tensor_scalar(
        d[:], s[:], math.sqrt(2.0) / 2.0, 2.0 * math.sqrt(2.0) * EPS,
        ALU.mult, ALU.add,
    )
    c0 = sb.tile([2 * N, 2], F32, tag="c0")
    nc.gpsimd.tensor_tensor(c0[:], fn[:, 0:2], fn[:, 2:4], ALU.add)  # -(lo+hi)
    r2 = sb.tile([2 * N, 2], F32, tag="r2")
    nc.vector.reciprocal(r2[:], d[:])
    # all-zero box -> factor-poisoning penalty via the exp bias vectors
    sp_ = sb.tile([2 * N, 2], F32, tag="sp_")
    nc.gpsimd.tensor_tensor(sp_[:], bbt[:, 0:2], bbt[:, 2:4], ALU.add)
    s4 = sb.tile([2 * N, 1], F32, tag="s4")
    nc.gpsimd.tensor_tensor(s4[:], sp_[:, 0:1], sp_[:, 1:2], ALU.add)
    pz = sb.tile([2 * N, 1], F32, tag="pz")
    nc.gpsimd.tensor_scalar(pz[:], s4[:], 0.0, None, ALU.is_equal)
    bv0 = sb.tile([2 * N, 1], F32, tag="bv0")
    nc.gpsimd.tensor_scalar(bv0[:], pz[:], -1.4e6, CL, ALU.mult, ALU.add)
    # e28 is emitted at half log-scale (2^14.5) so e56 = e28^2 directly
    bv1 = sb.tile([2 * N, 1], F32, tag="bv1")
    nc.gpsimd.tensor_scalar(bv1[:], pz[:], -2.8e6, CL / 2.0, ALU.mult, ALU.add)

    # t = (2j - lo - hi) / (2*sqrt2*(0.25*(hi-lo) + eps));  u = t^2
    t = sb.tile([2 * N, 2 * W], F32, tag="t")
    nc.vector.tensor_scalar(
        t[:, 0:W], iota2[:], c0[:, 1:2], r2[:, 1:2], ALU.add, ALU.mult
    )
    nc.vector.tensor_scalar(
        t[:, W : 2 * W], iota2[:], c0[:, 0:1], r2[:, 0:1], ALU.add, ALU.mult
    )
    u = sb.tile([2 * N, 2 * W], F32, tag="u")
    nc.gpsimd.tensor_tensor(u[:], t[:], t[:], ALU.mult)

    # factors bf16 (quantization shrinks by the 1/112 root)
    e1 = sb.tile([2 * N, 2 * W], BF16, tag="e1")
    nc.scalar.activation(e1[:], u[:], ACT.Exp, bias=bv0[:], scale=-14.0)
    e2 = sb.tile([2 * N, 2 * W], BF16, tag="e2")
    nc.scalar.activation(e2[:], u[:], ACT.Exp, bias=bv1[:], scale=-28.0)
    # e56 = e28^2 on DVE (bf16 2x mode)
    e4 = sb.tile([2 * N, 2 * W], BF16, tag="e4")
    nc.vector.tensor_tensor(e4[:], e2[:], e2[:], ALU.mult)

    # one PSUM bank (512 f32) per matmul group: HW rejects two accumulation
    # groups in one bank (CoreSim does not model this). 6 banks, single-buffered.
    ps = pp.tile([128, 6 * BK], F32, tag="ps", bufs=1)
    for lv, et in enumerate((e1, e2, e4)):
        for b in range(B_LOC):
            nc.tensor.matmul(
                ps[:, (2 * lv + b) * BK : (2 * lv + b) * BK + W],
                et[32 * b : 32 * (b + 1), 0:W],          # y-factors (lhsT)
                et[32 * b : 32 * (b + 1), W : 2 * W],    # x-factors (rhs)
                start=True,
                stop=True,
            )
    return ps


def _phase_b(nc, sb, att, ps, b0c, ebc):
    # L = fp16(ln(S + 1e-30)); [L14 | L28 | L56] blocks of 256
    L = sb.tile([128, 6 * W], F16, tag="L")
    psv = ps[:].rearrange("p (q c) -> p q c", q=6)[:, :, 0:W]
    nc.scalar.activation(L[:], psv, ACT.Ln, bias=b0c[:], scale=1.0)
    L1 = L[:, 0 : 2 * W]
    L2 = L[:, 2 * W : 4 * W]
    L4 = L[:, 4 * W : 6 * W]

    # min-cascade in 56*log domain (all fp16, 2x/4x DVE modes)
    a2 = sb.tile([128, 2 * W], F16, tag="a2")
    nc.vector.tensor_scalar(a2[:], L2, 2.0, None, ALU.mult)
    dd = sb.tile([128, 2 * W], F16, tag="dd")
    nc.vector.tensor_tensor(dd[:], a2[:], L4, ALU.subtract)
    dc = sb.tile([128, 2 * W], F16, tag="dc")
    nc.vector.tensor_scalar(dc[:], dd[:], 0.0, LN2, ALU.max, ALU.min)
    h = sb.tile([128, 2 * W], F16, tag="h")
    nc.vector.tensor_scalar(h[:], dc[:], PC2, PC1, ALU.mult, ALU.add)
    g = sb.tile([128, 2 * W], F16, tag="g")
    nc.vector.tensor_tensor(g[:], h[:], dc[:], ALU.mult)
    a4 = sb.tile([128, 2 * W], F16, tag="a4")
    nc.vector.tensor_tensor(a4[:], L4, g[:], ALU.add)
    c14 = sb.tile([128, 2 * W], F16, tag="c14")
    nc.vector.tensor_scalar(c14[:], L1, 4.0, -3.0 * K58, ALU.mult, ALU.add)
    m1 = sb.tile([128, 2 * W], F16, tag="m1")
    nc.vector.tensor_tensor(m1[:], a4[:], a2[:], ALU.min)
    marg = sb.tile([128, 2 * W], F16, tag="marg")
    nc.vector.tensor_tensor(marg[:], m1[:], c14[:], ALU.min)

    res = sb.tile([128, 2 * W], F32, tag="res")
    nc.scalar.activation(
        res[:], marg[:], ACT.Exp, bias=ebc[:], scale=1.0 / 56.0
    )
    nc.sync.dma_start(
        att.ap().rearrange("b y x -> y b x"),
        res[:].rearrange("p (b x) -> p b x", b=B_LOC),
    )


def _get_nc():
    if "nc" not in _CACHE:
        _CACHE["nc"] = build_nc()
    return _CACHE["nc"]


def kernel(feature_map: np.ndarray, bboxes: np.ndarray) -> np.ndarray:
    nc = _get_nc()
    bb = np.ascontiguousarray(bboxes, dtype=np.float32)
    in_maps = [
        {"bb": bb[c * B_LOC : (c + 1) * B_LOC]} for c in range(N_CORES)
    ]
    res = run_bass_kernel_spmd(nc, in_maps, list(range(N_CORES)))
    out = np.concatenate([res.results[c]["att"] for c in range(N_CORES)], axis=0)
    return out[:, None, :, :].astype(np.float32, copy=False)
